# revision 2
# baseline (speedup 1.0000x reference)
"""Trainium2 Bass kernel for nn_Decoder: 16-step GRU decoder with vocab-parallel
tensor sharding across 8 NeuronCores.

Sharding: vocab dim (V=32000, padded to 32768) split 4096/core. Per step:
  - logits shard (Vs,B) = w_out_shard @ h1  (weight-stationary, bf16, N=B=128)
  - softmax over batch axis = free-axis reduction, core-local
  - x partial = probsT.T @ w_inT (activation-stationary, N=512) -> bf16 AllReduce
    -> PE-transpose back to (H,B)
  - GRU hidden dim H=1024 split 128/core; AllGather h0, h1 (bf16)
Emission order fills collective latency with the next step's h-dependent
matmuls plus optional "heater" matmuls that keep the PE HAM clock at 2.4 GHz.
"""
import numpy as np
import ml_dtypes

import concourse.bass as bass
import concourse.mybir as mybir
import concourse.tile as tile
from concourse import bacc
from concourse import bass_utils

F32 = mybir.dt.float32
BF16 = mybir.dt.bfloat16
AF = mybir.ActivationFunctionType

B = 128
H = 1024
V = 32000
T = 16
BOS = 1
NC = 8
VS = 4096          # per-core padded vocab
VT = VS // 128     # 32 vocab tiles per core
KH = H // 128      # 8 H k-tiles
VPAD = NC * VS     # 32768


def build_nc(steps=T, n_cores=NC, heat_ar=0, heat_ag0=0, heat_ag1=0,
             heat_sm=0, split_banks=False, hoist=True, XGAP=32,
             enable_asserts=False):
    nc = bacc.Bacc("TRN2", target_bir_lowering=False, debug=False,
                   num_devices=n_cores, enable_asserts=enable_asserts)
    rg = [list(range(n_cores))]
    lg_bufs = 1 if split_banks else 2
    tp_bufs = 1 if split_banks else 2

    # ---- DRAM I/O ----
    d_woutT = nc.dram_tensor("woutT", [128, KH, VS], BF16, kind="ExternalInput").ap()
    d_bout = nc.dram_tensor("bout", [128, VT], F32, kind="ExternalInput").ap()
    d_winT = nc.dram_tensor("winT", [128, VT, H], BF16, kind="ExternalInput").ap()
    d_g = {}
    for nm in ("gih0", "ghh0", "gih1", "ghh1"):
        d_g[nm] = nc.dram_tensor(nm, [128, KH, 384], BF16, kind="ExternalInput").ap()
    d_brz0 = nc.dram_tensor("brz0", [128, 2], F32, kind="ExternalInput").ap()
    d_bn0 = nc.dram_tensor("bn0", [128, 2], F32, kind="ExternalInput").ap()
    d_brz1 = nc.dram_tensor("brz1", [128, 2], F32, kind="ExternalInput").ap()
    d_bn1 = nc.dram_tensor("bn1", [128, 2], F32, kind="ExternalInput").ap()
    d_x0T = nc.dram_tensor("x0T", [128, KH, B], BF16, kind="ExternalInput").ap()
    d_h0f = nc.dram_tensor("h0f", [128, KH, B], F32, kind="ExternalInput").ap()
    d_h1f = nc.dram_tensor("h1f", [128, KH, B], F32, kind="ExternalInput").ap()
    d_h0own = nc.dram_tensor("h0own", [128, B], F32, kind="ExternalInput").ap()
    d_h1own = nc.dram_tensor("h1own", [128, B], F32, kind="ExternalInput").ap()
    d_ident = nc.dram_tensor("ident", [128, 128], BF16, kind="ExternalInput").ap()
    d_out = nc.dram_tensor("logits", [steps, VT, 128, B], F32, kind="ExternalOutput").ap()

    with tile.TileContext(nc) as tc:
        with tc.tile_pool(name="wpool", bufs=1) as wpool, \
             tc.tile_pool(name="state", bufs=1) as state, \
             tc.tile_pool(name="sb", bufs=3) as sb, \
             tc.tile_pool(name="ps", bufs=1, space="PSUM") as ps, \
             tc.tile_pool(name="dram", bufs=2, space="DRAM") as dram:

            # ---- load weights to SBUF (resident); GRU weights first so the
            # first step's gate matmuls start without waiting on the 16MB of
            # vocab weights ----
            g_sb = {}
            for nm in ("gih0", "ghh0", "gih1", "ghh1"):
                t_ = wpool.tile([128, KH, 384], BF16, name=nm + "_sb")
                nc.sync.dma_start(t_[:], d_g[nm][:])
                g_sb[nm] = t_
            bout_sb = wpool.tile([128, VT], F32)
            nc.sync.dma_start(bout_sb[:], d_bout[:])
            brz = [wpool.tile([128, 2], F32, name=f"brz{l}_sb") for l in range(2)]
            bn = [wpool.tile([128, 2], F32, name=f"bn{l}_sb") for l in range(2)]
            nc.sync.dma_start(brz[0][:], d_brz0[:])
            nc.sync.dma_start(brz[1][:], d_brz1[:])
            nc.sync.dma_start(bn[0][:], d_bn0[:])
            nc.sync.dma_start(bn[1][:], d_bn1[:])
            ident = wpool.tile([128, 128], BF16)
            nc.sync.dma_start(ident[:], d_ident[:])

            x0_sb = state.tile([128, KH, B], BF16)
            nc.sync.dma_start(x0_sb[:], d_x0T[:])
            warm_sb = sb.tile([128, 2], BF16, tag="ccwarm", bufs=1)
            nc.vector.tensor_copy(out=warm_sb[:], in_=ident[:, 0:2])
            warm_in = dram.tile([128, 2], BF16, tag="ccwin", bufs=1)
            warm_ago = dram.tile([n_cores * 128, 2], BF16, tag="ccwago", bufs=1)
            warm_aro = dram.tile([128, 2], BF16, tag="ccwaro", bufs=1)
            nc.sync.dma_start(warm_in[:], warm_sb[:])
            nc.gpsimd.collective_compute(
                "AllGather", mybir.AluOpType.bypass, replica_groups=rg,
                ins=[warm_in.opt()], outs=[warm_ago.opt()])
            nc.gpsimd.collective_compute(
                "AllReduce", mybir.AluOpType.add, replica_groups=rg,
                ins=[warm_in.opt()], outs=[warm_aro.opt()])
            wout_sb = wpool.tile([128, KH, VS], BF16)
            for k in range(KH):
                nc.sync.dma_start(wout_sb[:, k, :], d_woutT[:, k, :])
            win_sb = wpool.tile([128, VT, H], BF16)
            for vo in range(VT):
                nc.sync.dma_start(win_sb[:, vo, :], d_winT[:, vo, :])

            # ---- hidden state init ----
            hf = []    # full hidden, bf16 [128, KH, B]
            hown = []  # own chunk fp32 [128, B]
            for l, (dfull, downn) in enumerate(((d_h0f, d_h0own), (d_h1f, d_h1own))):
                tmp = sb.tile([128, KH, B], F32, tag="lstage", bufs=1, name=f"hinit{l}")
                nc.sync.dma_start(tmp[:], dfull[:])
                fb = state.tile([128, KH, B], BF16, name=f"h{l}fb")
                nc.vector.tensor_copy(out=fb[:], in_=tmp[:])
                hf.append(fb)
                own = state.tile([128, B], F32, name=f"h{l}own")
                nc.sync.dma_start(own[:], downn[:])
                hown.append(own)

            xgb = state.tile([128, H], BF16)      # gathered x, (B,H) layout
            xbf = state.tile([128, KH, B], BF16)  # x transposed, (H,B) layout

            def heat(n, t, where, anchor):
                # junk matmuls that keep the PE HAM activity window busy during
                # collectives; anchored (via lhsT) to a tile written right at
                # gap start so the scheduler places them inside the gap
                for i in range(n):
                    hps = ps.tile([128, 512], F32, tag="lg", bufs=lg_bufs,
                                  name=f"heat_{where}_{t}_{i}")
                    nc.tensor.matmul(hps[:], anchor, x0_sb[:, 0:4, :],
                                     start=True, stop=True)

            hsl = [None, None]  # SBUF copy of the hh-side gate pre-activations
            hbl = [None, None]  # bf16 cast of own h chunk (AllGather input)

            def emit_ghh(l, t):
                """h-dependent half of the gate matmuls (ready early; fills
                collective gaps). Own bank; every accumulation group closes
                before the next starts (start=True clears the WHOLE bank's
                has_written bits), then drains to SBUF so the bank recycles."""
                g_h = ps.tile([128, 384], F32, tag="gh", name=f"gh{l}_{t}")
                ghh = g_sb[f"ghh{l}"]
                for g in range(3):  # r | z | hn
                    for k in range(KH):
                        nc.tensor.matmul(g_h[:, g * 128:(g + 1) * 128],
                                         ghh[:, k, g * 128:(g + 1) * 128],
                                         hf[l][:, k, :], start=(k == 0),
                                         stop=(k == KH - 1))
                hs = sb.tile([128, 384], F32, tag=f"hs{l}", bufs=1, name=f"hs{l}_{t}")
                nc.vector.tensor_copy(out=hs[:], in_=g_h[:])
                hsl[l] = hs

            def emit_gih_ew_ag(l, t, x_rhs_tiles):
                hs = hsl[l]
                g_x = ps.tile([128, 384], F32, tag="gx", name=f"gx{l}_{t}")
                gih = g_sb[f"gih{l}"]
                for g in range(3):  # r | z | ni
                    for k in range(KH):
                        nc.tensor.matmul(g_x[:, g * 128:(g + 1) * 128],
                                         gih[:, k, g * 128:(g + 1) * 128],
                                         x_rhs_tiles[k], start=(k == 0),
                                         stop=(k == KH - 1))
                # elementwise
                rzsum = sb.tile([128, 256], F32, tag="ew_rz", bufs=2, name=f"rz{l}_{t}")
                nc.vector.tensor_add(out=rzsum[:], in0=hs[:, 0:256], in1=g_x[:, 0:256])
                r = sb.tile([128, B], F32, tag="ew_r", bufs=2, name=f"r{l}_{t}")
                nc.scalar.activation(r[:], rzsum[:, 0:128], AF.Sigmoid, bias=brz[l][:, 0:1])
                z = sb.tile([128, B], F32, tag="ew_z", bufs=2, name=f"z{l}_{t}")
                nc.scalar.activation(z[:], rzsum[:, 128:256], AF.Sigmoid, bias=brz[l][:, 1:2])
                rhn = sb.tile([128, B], F32, tag="ew_rhn", bufs=2, name=f"rhn{l}_{t}")
                nc.vector.scalar_tensor_tensor(
                    out=rhn[:], in0=hs[:, 256:384], scalar=bn[l][:, 1:2], in1=r[:],
                    op0=mybir.AluOpType.add, op1=mybir.AluOpType.mult)
                pre = sb.tile([128, B], F32, tag="ew_pre", bufs=2, name=f"pre{l}_{t}")
                nc.vector.tensor_add(out=pre[:], in0=rhn[:], in1=g_x[:, 256:384])
                n = sb.tile([128, B], F32, tag="ew_n", bufs=2, name=f"n{l}_{t}")
                nc.scalar.activation(n[:], pre[:], AF.Tanh, bias=bn[l][:, 0:1])
                s = sb.tile([128, B], F32, tag="ew_s", bufs=2, name=f"s{l}_{t}")
                nc.vector.tensor_sub(out=s[:], in0=hown[l][:], in1=n[:])
                zs = sb.tile([128, B], F32, tag="ew_zs", bufs=2, name=f"zs{l}_{t}")
                nc.vector.tensor_mul(out=zs[:], in0=z[:], in1=s[:])
                nc.vector.tensor_add(out=hown[l][:], in0=n[:], in1=zs[:])
                # cast + allgather
                hb = sb.tile([128, B], BF16, tag="agc", bufs=2, name=f"agc{l}_{t}")
                nc.vector.tensor_copy(out=hb[:], in_=hown[l][:])
                hbl[l] = hb
                agin = dram.tile([128, B], BF16, tag=f"agin{l}", name=f"agin{l}_{t}")
                agout = dram.tile([n_cores * 128, B], BF16, tag=f"agout{l}",
                                  name=f"agout{l}_{t}")
                nc.sync.dma_start(agin[:], hb[:])
                nc.gpsimd.collective_compute(
                    "AllGather", mybir.AluOpType.bypass, replica_groups=rg,
                    ins=[agin.opt()], outs=[agout.opt()])
                nc.sync.dma_start(
                    hf[l][:], agout.rearrange("(ko ki) b -> ki ko b", ki=128))

            # hh matmuls for step 0 are ready immediately
            if hoist:
                emit_ghh(0, 0)
                emit_ghh(1, 0)

            for t in range(steps):
                x_rhs = [x0_sb[:, k, :] for k in range(KH)] if t == 0 \
                    else [xbf[:, k, :] for k in range(KH)]
                if not hoist:
                    emit_ghh(0, t)
                emit_gih_ew_ag(0, t, x_rhs)
                heat(heat_ag0, t, "ag0", hbl[0][:])
                if not hoist:
                    emit_ghh(1, t)
                emit_gih_ew_ag(1, t, [hf[0][:, k, :] for k in range(KH)])
                if hoist and t + 1 < steps:
                    emit_ghh(0, t + 1)       # fills AG1(t) latency
                heat(heat_ag1, t, "ag1", hbl[1][:])

                # ---- logits + softmax ----
                last = (t == steps - 1)
                probs = []
                if not last:
                    sums = sb.tile([128, VT], F32, tag="sums", bufs=2, name=f"sums_{t}")
                    recs = sb.tile([128, VT], F32, tag="recs", bufs=2, name=f"recs_{t}")
                lstage = sb.tile([128, VT, B], F32, tag="lstage", bufs=1,
                                 name=f"lstage_{t}")
                if not last:
                    xpA = ps.tile([128, 512], F32, tag="xpA", name=f"xpA_{t}")
                    xpB = ps.tile([128, 512], F32, tag="xpB", name=f"xpB_{t}")

                def emit_xpart(vo):
                    # x partial (activation-stationary, out (B,H)); interleaved
                    # into the logits loop two tiles behind the softmax chain
                    nc.tensor.matmul(xpA[:], probs[vo][:], win_sb[:, vo, 0:512],
                                     start=(vo == 0), stop=(vo == VT - 1))
                    nc.tensor.matmul(xpB[:], probs[vo][:], win_sb[:, vo, 512:1024],
                                     start=(vo == 0), stop=(vo == VT - 1))

                for j in range(VT):
                    lg = ps.tile([128, B], F32, tag="lg", bufs=lg_bufs, name=f"lg_{t}_{j}")
                    for k in range(KH):
                        nc.tensor.matmul(lg[:], wout_sb[:, k, j * 128:(j + 1) * 128],
                                         hf[1][:, k, :], start=(k == 0), stop=(k == KH - 1))
                    nc.vector.tensor_scalar_add(lstage[:, j, :], lg[:], bout_sb[:, j:j + 1])
                    if not last:
                        pe = sb.tile([128, B], BF16, tag="probs", bufs=VT,
                                     name=f"probs_{t}_{j}")
                        nc.scalar.activation(pe[:], lg[:], AF.Exp,
                                             bias=bout_sb[:, j:j + 1],
                                             accum_out=sums[:, j:j + 1])
                        nc.vector.reciprocal(recs[:, j:j + 1], sums[:, j:j + 1])
                        nc.vector.tensor_scalar_mul(pe[:], pe[:], recs[:, j:j + 1])
                        probs.append(pe)
                        if j >= XGAP:
                            emit_xpart(j - XGAP)

                nc.sync.dma_start(
                    d_out[t].rearrange("j vi b -> vi j b"), lstage[:])

                if not last:
                    heat(heat_sm, t, "sm", probs[VT - 8][:])
                    for vo in range(VT - XGAP, VT):
                        emit_xpart(vo)
                    xstage = sb.tile([128, H], BF16, tag="xstage", bufs=2,
                                     name=f"xstage_{t}")
                    nc.vector.tensor_copy(out=xstage[:, 0:512], in_=xpA[:])
                    nc.vector.tensor_copy(out=xstage[:, 512:1024], in_=xpB[:])
                    arin = dram.tile([128, H], BF16, tag="arin", name=f"arin_{t}")
                    arout = dram.tile([128, H], BF16, tag="arout", name=f"arout_{t}")
                    nc.sync.dma_start(arin[:], xstage[:])
                    nc.gpsimd.collective_compute(
                        "AllReduce", mybir.AluOpType.add, replica_groups=rg,
                        ins=[arin.opt()], outs=[arout.opt()])
                    if hoist:
                        emit_ghh(1, t + 1)   # fills AllReduce latency
                    heat(heat_ar, t, "ar", xstage[:, 0:128])
                    nc.sync.dma_start(xgb[:], arout[:])
                    # transpose (B,H) -> (H,B) via PE
                    for m in range(KH):
                        tp = ps.tile([128, 128], BF16, tag="tp", bufs=tp_bufs,
                                     name=f"tp_{t}_{m}")
                        nc.tensor.transpose(tp[:], xgb[:, m * 128:(m + 1) * 128],
                                            ident[:])
                        nc.vector.tensor_copy(out=xbf[:, m, :], in_=tp[:])

    nc.compile()
    return nc


# ---------------- host side ----------------

def _prep_core_inputs(c, hidden, w_in, b_in, W_ih0, W_hh0, b_ih0, b_hh0,
                      W_ih1, W_hh1, b_ih1, b_hh1, w_out, b_out):
    bf = ml_dtypes.bfloat16
    w_inT_pad = np.zeros((VPAD, H), np.float32)
    w_inT_pad[:V] = w_in.T
    w_outT_pad = np.zeros((H, VPAD), np.float32)
    w_outT_pad[:, :V] = w_out.T
    b_out_pad = np.zeros(VPAD, np.float32)
    b_out_pad[:V] = b_out

    d = {}
    wv = w_outT_pad[:, c * VS:(c + 1) * VS]              # (H, VS)
    d["woutT"] = np.ascontiguousarray(
        wv.reshape(KH, 128, VS).transpose(1, 0, 2)).astype(bf)
    d["bout"] = np.ascontiguousarray(
        b_out_pad[c * VS:(c + 1) * VS].reshape(VT, 128).T)
    winv = w_inT_pad[c * VS:(c + 1) * VS, :]              # (VS, H)
    d["winT"] = np.ascontiguousarray(
        winv.reshape(VT, 128, H).transpose(1, 0, 2)).astype(bf)

    sel = np.concatenate([np.arange(c * 128, (c + 1) * 128) + g * H for g in range(3)])
    for nm, W in (("gih0", W_ih0), ("ghh0", W_hh0), ("gih1", W_ih1), ("ghh1", W_hh1)):
        Wsel = W[sel]                                     # (384, H)
        d[nm] = np.ascontiguousarray(
            Wsel.T.reshape(KH, 128, 384).transpose(1, 0, 2)).astype(bf)

    for l, (W_ih, b_ih, b_hh) in enumerate(((W_ih0, b_ih0, b_hh0),
                                            (W_ih1, b_ih1, b_hh1))):
        ih_eff = b_ih[sel].astype(np.float32)
        if l == 0:
            ih_eff = ih_eff + W_ih0[sel] @ b_in
        hh = b_hh[sel].astype(np.float32)
        d[f"brz{l}"] = np.stack([ih_eff[:128] + hh[:128],
                                 ih_eff[128:256] + hh[128:256]], axis=1)
        d[f"bn{l}"] = np.stack([ih_eff[256:384], hh[256:384]], axis=1)

    x0 = w_inT_pad[BOS]                                   # (H,) == w_in[:, BOS]
    d["x0T"] = np.ascontiguousarray(
        np.broadcast_to(x0.reshape(KH, 128).T[:, :, None], (128, KH, B))).astype(bf)

    for l in range(2):
        hT = hidden[l].T                                  # (H, B)
        d[f"h{l}f"] = np.ascontiguousarray(
            hT.reshape(KH, 128, B).transpose(1, 0, 2)).astype(np.float32)
        d[f"h{l}own"] = np.ascontiguousarray(
            hT[c * 128:(c + 1) * 128]).astype(np.float32)
    d["ident"] = np.eye(128, dtype=np.float32).astype(bf)
    return {k: np.ascontiguousarray(v) for k, v in d.items()}


_NC_CACHE = {}
BUILD_KW = dict(heat_ar=40, heat_ag0=20, heat_ag1=20, heat_sm=4)


def _get_nc(steps=T):
    key = (steps, tuple(sorted(BUILD_KW.items())))
    if key not in _NC_CACHE:
        _NC_CACHE[key] = build_nc(steps, **BUILD_KW)
    return _NC_CACHE[key]


def kernel(**inputs):
    nc = _get_nc(T)
    in_maps = [_prep_core_inputs(c, **inputs) for c in range(NC)]
    res = bass_utils.run_bass_kernel_spmd(nc, in_maps, core_ids=list(range(NC)))
    out_pad = np.zeros((T, B, VPAD), np.float32)
    for c in range(NC):
        o = res.results[c]["logits"]                      # (T, VT, 128, B)
        out_pad[:, :, c * VS:(c + 1) * VS] = o.transpose(0, 3, 1, 2).reshape(T, B, VS)
    return out_pad[:, :, :V]

